# revision 1
# baseline (speedup 1.0000x reference)
"""Trainium2 Bass kernel for GCN(+self-loops, sym-norm) + CBAM block.

Self-contained SPMD kernel over 8 NeuronCores, dst-node sharding with
lane-major node labels (node n: core=n//NSH, lane=(n%NSH)//NB, block=n%NB),
so every bulk DMA is 128 contiguous-KB packets.

  phase 0: x (host-cast bf16) loaded via HW transpose-DMA as a parity-split
           [128, NSH/2] xT; h' = (x@W)*dinv_src per block (bf16 PE matmul);
           h*dinv^2+bias self-loop term; hsh [NSH, 2C] (256B-stride rows).
  phase 1: 4 chunked AllGathers -> per-quarter tables hfq[q]; AG1-3 triggers
           interleaved into the gather loop (cc enqueue blocks GPSIMD until
           the previous AG completes, so don't issue them back-to-back).
  phase 2: quarter-major SWDGE dma_gather (4 queues rotate over supergroups);
           onehot(dst lane) via bf16 iota is_equal; PE matmul-accumulates all
           of a supergroup's blocks in ONE PSUM bank per (sg,q) pass (single
           start=True; has_written gives per-block init), one merge-add per
           pass; per-block finalize fuses dinv_dst + self-loop and streams
           channel-sum (PE) / channel-max (DVE) stats.
  phase 3: tiny stats AllGather, CBAM channel MLP.
  phase 4: channel/spatial gates, residual, relu, contiguous store.
"""

import sys

for _p in ("/opt/trn_rl_repo", "/root/.axon_site/_ro/trn_rl_repo"):
    if _p not in sys.path:
        sys.path.insert(0, _p)

from contextlib import ExitStack

import numpy as np
import ml_dtypes

import concourse.bass as bass
import concourse.tile as tile
from concourse import bacc, mybir
from concourse.bass import AP
from concourse.bass_utils import run_bass_kernel_spmd
from concourse.masks import make_identity

P = 128
F32 = mybir.dt.float32
BF16 = mybir.dt.bfloat16
I16 = mybir.dt.int16
AF = mybir.ActivationFunctionType
ALU = mybir.AluOpType

N_CORES = 8


def _patch_dma_gather():
    """Relax the elem_size %256B assert for non-transpose gathers (the Q7
    ucode only requires the row STRIDE to be a 256B multiple)."""
    import inspect
    import textwrap

    if getattr(bass.BassGpSimd.dma_gather, "_elem_patch", False):
        return
    src = inspect.getsource(bass.BassGpSimd.dma_gather)
    src = src.replace(
        "assert (\n            elem_size_bytes > 0 and elem_size_bytes"
        " % 256 == 0\n        )  # transpose restriction",
        "assert elem_size_bytes > 0 and"
        " (not transpose or elem_size_bytes % 256 == 0)")
    ns = dict(bass.BassGpSimd.dma_gather.__globals__)
    exec(textwrap.dedent(src), ns)
    ns["dma_gather"]._elem_patch = True
    bass.BassGpSimd.dma_gather = ns["dma_gather"]


_patch_dma_gather()


def mid_bcast(ap2d: AP, n: int) -> AP:
    (pstep, pcnt), (istep, icnt) = ap2d.ap
    return AP(ap2d.tensor, ap2d.offset, [[pstep, pcnt], [0, n], [istep, icnt]])


def view3(ap2d: AP, d1: int, d2: int, transpose=False) -> AP:
    (pstep, pcnt), (istep, icnt) = ap2d.ap
    assert icnt == d1 * d2
    if transpose:
        return AP(ap2d.tensor, ap2d.offset,
                  [[pstep, pcnt], [istep, d2], [istep * d2, d1]])
    return AP(ap2d.tensor, ap2d.offset,
              [[pstep, pcnt], [istep * d2, d1], [istep, d2]])


IND_FRAC = 0.0   # fraction of tiles gathered via indirect_dma_start


def preprocess(edge_index: np.ndarray, N: int, n_cores: int, sg_max_tiles=160):
    NB = -(-N // (n_cores * P))
    NSH = NB * P
    NPAD = n_cores * NSH
    CH = NSH // 4          # rows per core per chunk
    QROW = n_cores * CH    # table rows per chunk

    src_f = np.asarray(edge_index[0], dtype=np.int64)
    dst_f = np.asarray(edge_index[1], dtype=np.int64)
    deg = np.bincount(dst_f, minlength=NPAD).astype(np.float32)
    deg[:N] += 1.0
    dinv = np.zeros(NPAD, dtype=np.float32)
    nz = deg > 0
    dinv[nz] = 1.0 / np.sqrt(deg[nz])

    core = dst_f // NSH
    dl = dst_f - core * NSH
    lane = (dl // NB).astype(np.float32)   # onehot lane id
    blk = dl % NB
    sl = src_f % NSH
    q = sl // CH
    rq = (src_f // NSH) * CH + (sl % CH)   # row within chunk table

    key = (q * n_cores + core) * NB + blk
    order = np.argsort(key, kind="stable")
    key_s = key[order]
    rq_s = rq[order]
    lane_s = lane[order]
    counts = np.bincount(key_s, minlength=4 * n_cores * NB)
    grp_start = np.concatenate([[0], np.cumsum(counts)[:-1]])
    cnt = counts.reshape(4, n_cores, NB)

    cap = -(-cnt.max(axis=1) // P) * P      # [4, NB] shared capacities
    blk_tiles = cap.sum(axis=0) // P        # [NB] tiles per block (all 4 q)

    # supergroups: greedy pack blocks (<=8 blocks so the per-(sg,q) PSUM
    # accumulator [P, nblk*C] f32 fits one 2KB PSUM bank)
    sgs, cur, cur_t = [], [], 0
    for b in range(NB):
        t = int(blk_tiles[b])
        if cur and (cur_t + t > sg_max_tiles or len(cur) >= 8):
            sgs.append(cur)
            cur, cur_t = [], 0
        cur.append(b)
        cur_t += t
    if cur:
        sgs.append(cur)

    # slot layout: quarter-major, then sg order, then block
    seg_pos = np.zeros((4, NB), np.int64)
    sg_meta = []   # per sg: list of (q, s0, nq, [(b, tcols)...], k_ind, ind_off)
    pos = 0
    ind_pos = 0
    per_sg = [[] for _ in sgs]
    for qq in range(4):
        for si, sg in enumerate(sgs):
            s0 = pos
            blocks = []
            for b in sg:
                nsl = int(cap[qq, b])
                seg_pos[qq, b] = pos
                if nsl:
                    blocks.append((b, list(range(pos // P, (pos + nsl) // P))))
                pos += nsl
            ntile = (pos - s0) // P
            k_ind = int(ntile * IND_FRAC) if ntile >= 3 else 0
            per_sg[si].append((qq, s0, pos - s0, blocks, k_ind, ind_pos))
            ind_pos += k_ind
    TOT = pos
    TTI = max(ind_pos, 1)
    total_tiles = TOT // P
    for si in range(len(sgs)):
        sg_meta.append(per_sg[si])

    idx_wraps, dstl_arrs, idx32_arrs = [], [], []
    for c in range(n_cores):
        idx_flat = np.zeros(TOT, np.int32)
        dstl_flat = np.full(TOT, -1.0, np.float32)
        for qq in range(4):
            for b in range(NB):
                n_e = int(cnt[qq, c, b])
                capn = int(cap[qq, b])
                if capn == 0:
                    continue
                s0 = seg_pos[qq, b]
                if n_e:
                    g0 = grp_start[(qq * n_cores + c) * NB + b]
                    vals = rq_s[g0:g0 + n_e].astype(np.int32)
                    idx_flat[s0:s0 + n_e] = vals
                    dstl_flat[s0:s0 + n_e] = lane_s[g0:g0 + n_e]
                    if n_e < capn:
                        idx_flat[s0 + n_e:s0 + capn] = vals[-1]
        w = idx_flat.astype(np.int16).reshape(TOT // 16, 16).T
        iw = np.zeros((P, TOT // 16), np.int16)
        for r in range(8):
            iw[r * 16:(r + 1) * 16, :] = w
        idx_wraps.append(iw)
        dstl_arrs.append(dstl_flat.reshape(total_tiles, P).T.copy())
        # int32 idx for the indirect share: [P, TTI]; col j of (sg,q) block
        # k_ind at ind_off -> slot (s0 + (ntile-k_ind+j)*P + p)
        i32 = np.zeros((P, TTI), np.int32)
        for per_q in per_sg:
            for (qq, s0, nq, blocks, k_ind, ind_off) in per_q:
                if k_ind == 0:
                    continue
                ntile = nq // P
                base = s0 + (ntile - k_ind) * P
                blk32 = idx_flat[base:base + k_ind * P].reshape(k_ind, P).T
                i32[:, ind_off:ind_off + k_ind] = blk32
        idx32_arrs.append(i32)

    # dinv_dst [core][P, NB] lane-major: node = c*NSH + p*NB + b
    node_ids = (
        np.arange(n_cores)[:, None, None] * NSH
        + np.arange(P)[None, :, None] * NB
        + np.arange(NB)[None, None, :]
    )
    dinv_dst = dinv[node_ids].astype(np.float32)

    return dict(NB=NB, NSH=NSH, NPAD=NPAD, CH=CH, QROW=QROW, sgs=sgs,
                total_tiles=total_tiles, TTI=TTI, sg_meta=sg_meta,
                idx_wraps=idx_wraps, dstl_arrs=dstl_arrs,
                idx32_arrs=idx32_arrs, dinv=dinv, dinv_dst=dinv_dst)


def build_nc(meta, n_cores: int, N: int, C: int, H: int, debug: bool = False):
    NB, NSH, NPAD = meta["NB"], meta["NSH"], meta["NPAD"]
    CH, QROW = meta["CH"], meta["QROW"]
    TT = meta["total_tiles"]
    NBH = NB // 2

    nc = bacc.Bacc("TRN2", target_bir_lowering=False, debug=debug,
                   enable_asserts=False, num_devices=n_cores,
                   num_swdge_queues=4)

    TTI = meta["TTI"]
    I32 = mybir.dt.int32
    xs = nc.dram_tensor("xs", [NSH, C], BF16, kind="ExternalInput")
    idxw = nc.dram_tensor("idxw", [P, TT * 8], I16, kind="ExternalInput")
    idx32 = nc.dram_tensor("idx32", [P, TTI], I32, kind="ExternalInput")
    dstl = nc.dram_tensor("dstl", [P, TT], BF16, kind="ExternalInput")
    dinvd = nc.dram_tensor("dinvd", [P, NB], F32, kind="ExternalInput")
    W = nc.dram_tensor("W", [C, C], BF16, kind="ExternalInput")
    brow = nc.dram_tensor("brow", [1, C], F32, kind="ExternalInput")
    w1 = nc.dram_tensor("w1", [C, H], F32, kind="ExternalInput")
    w2 = nc.dram_tensor("w2", [H, C], F32, kind="ExternalInput")
    sprow = nc.dram_tensor("sprow", [1, 3], F32, kind="ExternalInput")
    out = nc.dram_tensor("out", [NSH, C], F32, kind="ExternalOutput")

    hsh = nc.dram_tensor("hsh", [NSH, 2 * C], BF16)
    hfq = [nc.dram_tensor(f"hfq{q}", [QROW, 2 * C], BF16, addr_space="Shared")
           for q in range(4)]
    stats_loc = nc.dram_tensor("stats_loc", [P, 1], F32)
    stats_ag = nc.dram_tensor("stats_ag", [P * n_cores, 1], F32,
                              addr_space="Shared")
    rg = [list(range(n_cores))]

    with tile.TileContext(nc) as tc, ExitStack() as ctx:
        const = ctx.enter_context(tc.tile_pool(name="const", bufs=1))
        big = ctx.enter_context(tc.tile_pool(name="big", bufs=1))
        pmisc = ctx.enter_context(tc.tile_pool(name="pmisc", bufs=2,
                                               space="PSUM"))

        # ---- constants ----
        ident = const.tile([P, P], F32)
        make_identity(nc, ident[:])
        iota_i = const.tile([P, P], mybir.dt.int32)
        nc.gpsimd.iota(iota_i[:], pattern=[[1, P]], channel_multiplier=0)
        iota_f = const.tile([P, P], BF16)
        nc.vector.tensor_copy(iota_f[:], iota_i[:])
        ones_row = const.tile([1, P], F32)
        nc.gpsimd.memset(ones_row[:], 1.0)
        ones_col = const.tile([P, 1], F32)
        nc.gpsimd.memset(ones_col[:], 1.0)
        ones2 = const.tile([2, 1], F32)
        nc.gpsimd.memset(ones2[:], 1.0)

        W_sb = const.tile([C, C], BF16)
        nc.sync.dma_start(W_sb[:], W.ap())
        # second copy of W on partitions 64..127 for odd-parity blocks
        W2_sb = const.tile([P, C], BF16)
        nc.sync.dma_start(W2_sb[0:C, :], W.ap())
        nc.sync.dma_start(W2_sb[C:2 * C, :], W.ap())
        brow_sb = const.tile([1, C], F32)
        nc.sync.dma_start(brow_sb[:], brow.ap())
        w1_sb = const.tile([C, H], F32)
        nc.sync.dma_start(w1_sb[:], w1.ap())
        w2_sb = const.tile([H, C], F32)
        nc.sync.dma_start(w2_sb[:], w2.ap())
        sprow_sb = const.tile([1, 3], F32)
        nc.sync.dma_start(sprow_sb[:], sprow.ap())

        bb_ps = pmisc.tile([P, C], F32, space="PSUM", tag="mm")
        nc.tensor.matmul(bb_ps[:], lhsT=ones_row[:], rhs=brow_sb[:],
                         start=True, stop=True)
        b_bc = const.tile([P, C], F32)
        nc.scalar.copy(b_bc[:], bb_ps[:])
        sp_ps = pmisc.tile([P, 3], F32, space="PSUM", tag="mm")
        nc.tensor.matmul(sp_ps[:], lhsT=ones_row[:], rhs=sprow_sb[:],
                         start=True, stop=True)
        sp_bc = const.tile([P, 3], F32)
        nc.scalar.copy(sp_bc[:], sp_ps[:])
        bt_ps = pmisc.tile([C, 1], F32, space="PSUM", tag="mm")
        nc.tensor.transpose(bt_ps[:], in_=brow_sb[:], identity=ident[:1, :1])
        bT = const.tile([C, 1], F32)
        nc.scalar.copy(bT[:], bt_ps[:])

        dinvd_sb = big.tile([P, NB], F32)
        nc.sync.dma_start(dinvd_sb[:], dinvd.ap())
        dinv2_sb = big.tile([P, NB], F32)
        nc.vector.tensor_tensor(out=dinv2_sb[:], in0=dinvd_sb[:],
                                in1=dinvd_sb[:], op=ALU.mult)

        hb2 = big.tile([P, NB * 2 * C], BF16)   # [h'(p,b) | 0] 256B rows
        nc.gpsimd.memset(hb2[:], 0.0)
        agg_sb = big.tile([P, NB * C], F32)
        nc.gpsimd.memset(agg_sb[:], 0.0)
        hslb = big.tile([P, NB * C], F32)       # h*dinv^2 + bias (self-loop)
        rmax_run = big.tile([P, C], F32)
        nc.gpsimd.memset(rmax_run[:], -3.0e38)

        # ---- phase 0: h' = (x @ W) * dinv, parity-split transpose load ----
        with tc.tile_pool(name="ph0", bufs=1) as ph0, \
             tc.tile_pool(name="pp0", bufs=4, space="PSUM") as pp0:
            xT2 = ph0.tile([P, NSH // 2], BF16)
            nc.sync.dma_start_transpose(
                xT2[:], AP(xs, 0, [[2 * C, NSH // 2], [1, 2 * C]]))
            for b in range(NB):
                beta = b % 2
                half = xT2[beta * C:(beta + 1) * C, :]
                lhsT = AP(half.tensor, half.offset + b // 2,
                          [[half.ap[0][0], C], [NBH, P]])
                h_ps = pp0.tile([P, C], F32, space="PSUM", tag="h")
                nc.tensor.matmul(h_ps[:], lhsT=lhsT,
                                 rhs=W2_sb[beta * C:(beta + 1) * C, :],
                                 start=True, stop=True)
                nc.vector.tensor_scalar(
                    out=hb2[:, b * 2 * C:b * 2 * C + C], in0=h_ps[:],
                    scalar1=dinvd_sb[:, b:b + 1], scalar2=None, op0=ALU.mult)
                nc.vector.tensor_scalar(
                    out=hslb[:, b * C:(b + 1) * C], in0=h_ps[:],
                    scalar1=dinv2_sb[:, b:b + 1], scalar2=None, op0=ALU.mult)
            nc.sync.dma_start(
                hsh.ap().rearrange("(p b) c -> p b c", p=P),
                view3(hb2[:], NB, 2 * C))
        # fold bias into the self-loop term (stats then include bias)
        nc.vector.tensor_tensor(out=view3(hslb[:], NB, C),
                                in0=view3(hslb[:], NB, C),
                                in1=mid_bcast(b_bc[:], NB), op=ALU.add)

        # ---- phase 1: chunked AllGathers (AG2/3 issued inside sg loop) ----
        def issue_ag(q):
            nc.gpsimd.collective_compute(
                "AllGather", ALU.bypass, replica_groups=rg,
                ins=[AP(hsh, q * CH * 2 * C, [[2 * C, CH], [1, 2 * C]])],
                outs=[hfq[q].ap()])

        issue_ag(0)

        # phase-2-only inputs: issue loads after the AG trigger chain
        idxw_sb = big.tile([P, TT * 8], I16)
        nc.sync.dma_start(idxw_sb[:], idxw.ap())
        dstl_sb = big.tile([P, TT], BF16)
        nc.sync.dma_start(dstl_sb[:], dstl.ap())
        idx32_sb = big.tile([P, TTI], mybir.dt.int32)
        nc.sync.dma_start(idx32_sb[:], idx32.ap())

        # ---- phase 2: quarter-major gather + aggregate ----
        qtab = [AP(hfq[q], 0, [[2 * C, QROW], [1, C]]) for q in range(4)]
        n_sg = len(meta["sg_meta"])
        stat_started = [False]
        with tc.tile_pool(name="gt", bufs=6) as gtp, \
             tc.tile_pool(name="oh", bufs=3) as ohp, \
             tc.tile_pool(name="pp2", bufs=4, space="PSUM") as pp2, \
             tc.tile_pool(name="pst", bufs=1, space="PSUM") as pst:
            stat_ps = pst.tile([1, 512], F32, space="PSUM", tag="stat")

            def finalize_block(b, last):
                # ab = agg*dinv + (h*dinv^2 + bias); then streaming stats
                nc.vector.scalar_tensor_tensor(
                    out=agg_sb[:, b * C:(b + 1) * C],
                    in0=agg_sb[:, b * C:(b + 1) * C],
                    scalar=dinvd_sb[:, b:b + 1], op0=ALU.mult,
                    in1=hslb[:, b * C:(b + 1) * C], op1=ALU.add)
                nc.tensor.matmul(
                    stat_ps[0:1, 0:C], lhsT=ones_col[:],
                    rhs=agg_sb[:, b * C:(b + 1) * C],
                    start=not stat_started[0], stop=last)
                stat_started[0] = True
                nc.vector.tensor_tensor(
                    out=rmax_run[:], in0=rmax_run[:],
                    in1=agg_sb[:, b * C:(b + 1) * C], op=ALU.max)

            for q in range(4):
                for si, sg in enumerate(meta["sg_meta"]):
                    qq, s0, nq, blocks, k_ind, ind_off = sg[q]
                    if nq == 0:
                        continue
                    ntile = nq // P
                    tb = s0 // P
                    nsw_t = ntile - k_ind
                    gath = gtp.tile([P, ntile * C], BF16, tag="gath")
                    if nsw_t:
                        nc.gpsimd.dma_gather(
                            out_ap=view3(gath[:, 0:nsw_t * C], nsw_t, C),
                            in_ap=qtab[q],
                            idxs_ap=idxw_sb[:, s0 // 16:(s0 + nsw_t * P) // 16],
                            num_idxs=nsw_t * P, num_idxs_reg=nsw_t * P,
                            elem_size=C, elem_step=2 * C,
                            single_packet=False, queue_num=si % 4)
                    if k_ind:
                        nc.gpsimd.indirect_dma_start(
                            out=view3(gath[:, nsw_t * C:], k_ind, C),
                            out_offset=None,
                            in_=hfq[q].ap(),
                            in_offset=bass.IndirectOffsetOnAxis(
                                ap=idx32_sb[:, ind_off:ind_off + k_ind],
                                axis=0),
                        )
                    onehot = ohp.tile([P, ntile * P], BF16, tag="oh")
                    nc.vector.tensor_tensor(
                        out=view3(onehot[:], ntile, P),
                        in0=dstl_sb[:, tb:tb + ntile].to_broadcast(
                            [P, ntile, P]),
                        in1=mid_bcast(iota_f[:], ntile),
                        op=ALU.is_equal)
                    b_first = blocks[0][0]
                    b_last = blocks[-1][0]
                    span = b_last - b_first + 1
                    agg_ps = pp2.tile([P, 8 * C], F32, space="PSUM",
                                      tag="agg")
                    nmm = sum(len(tc_) for _, tc_ in blocks)
                    k = 0
                    for b, tcols in blocks:
                        cb = (b - b_first) * C
                        for t in tcols:
                            nc.tensor.matmul(
                                agg_ps[:, cb:cb + C],
                                lhsT=onehot[:, (t - tb) * P:(t - tb + 1) * P],
                                rhs=gath[:, (t - tb) * C:(t - tb + 1) * C],
                                start=(k == 0), stop=(k == nmm - 1))
                            k += 1
                    # merge contiguous covered runs into agg_sb
                    covered = [b for b, _ in blocks]
                    ri = 0
                    while ri < len(covered):
                        rj = ri
                        while (rj + 1 < len(covered)
                               and covered[rj + 1] == covered[rj] + 1):
                            rj += 1
                        rb0, rbn = covered[ri], covered[rj] - covered[ri] + 1
                        nc.vector.tensor_tensor(
                            out=agg_sb[:, rb0 * C:(rb0 + rbn) * C],
                            in0=agg_sb[:, rb0 * C:(rb0 + rbn) * C],
                            in1=agg_ps[:, (rb0 - b_first) * C:
                                       (rb0 - b_first + rbn) * C],
                            op=ALU.add)
                        ri = rj + 1
                    if q == 3:
                        sgb = meta["sgs"][si]
                        for b in sgb:
                            finalize_block(
                                b, last=(si == n_sg - 1 and b == sgb[-1]))
                    if q == 0 and si == 0:
                        issue_ag(1)
                    if q == 0 and si == 5:
                        issue_ag(2)
                    if q == 0 and si == 9:
                        issue_ag(3)

            # ---- stats finalization (inside pools for stat_ps scope) ----
            stat_sb = const.tile([1, C], F32)
            nc.scalar.copy(stat_sb[:], stat_ps[0:1, 0:C])
            sT_ps = pmisc.tile([C, 1], F32, space="PSUM", tag="mm")
            nc.tensor.transpose(sT_ps[:], in_=stat_sb[:], identity=ident[:1, :1])
            loc = const.tile([P, 1], F32)
            nc.scalar.copy(loc[0:C, :], sT_ps[:])
            mT_ps = pmisc.tile([C, P], F32, space="PSUM", tag="mm")
            nc.tensor.transpose(mT_ps[:], in_=rmax_run[:], identity=ident[:])
            mT_sb = const.tile([C, P], F32)
            nc.scalar.copy(mT_sb[:], mT_ps[:])
            nc.vector.reduce_max(loc[C:2 * C, :], mT_sb[:],
                                 axis=mybir.AxisListType.X)
            nc.sync.dma_start(stats_loc.ap(), loc[:])
        nc.gpsimd.collective_compute(
            "AllGather", ALU.bypass, replica_groups=rg,
            ins=[stats_loc.ap()], outs=[stats_ag.ap()])
        ag_sb = const.tile([P, n_cores], F32)
        nc.sync.dma_start(ag_sb[:], AP(stats_ag, 0, [[1, P], [P, n_cores]]))
        gsum = const.tile([C, 1], F32)
        nc.vector.reduce_sum(gsum[:], ag_sb[0:C, :], axis=mybir.AxisListType.X)
        gmax_hi = const.tile([P, 1], F32)
        nc.vector.reduce_max(gmax_hi[C:2 * C, :], ag_sb[C:2 * C, :],
                             axis=mybir.AxisListType.X)
        gmax = const.tile([C, 1], F32)
        nc.sync.dma_start(gmax[:], gmax_hi[C:2 * C, :])

        v2 = const.tile([C, 2], F32)
        nc.vector.tensor_scalar(out=v2[:, 0:1], in0=gsum[:], scalar1=1.0 / N,
                                scalar2=None, op0=ALU.mult)
        nc.vector.tensor_copy(v2[:, 1:2], gmax[:])

        r1_ps = pmisc.tile([2, H], F32, space="PSUM", tag="mm")
        nc.tensor.matmul(r1_ps[:], lhsT=v2[:], rhs=w1_sb[:], start=True,
                         stop=True)
        r1_sb = const.tile([2, H], F32)
        nc.scalar.activation(r1_sb[:], r1_ps[:], func=AF.Relu)
        r1T_ps = pmisc.tile([H, 2], F32, space="PSUM", tag="mm")
        nc.tensor.transpose(r1T_ps[:], in_=r1_sb[:], identity=ident[:2, :2])
        r1T_sb = const.tile([H, 2], F32)
        nc.scalar.copy(r1T_sb[:], r1T_ps[:])
        r2_ps = pmisc.tile([2, C], F32, space="PSUM", tag="mm")
        nc.tensor.matmul(r2_ps[:], lhsT=r1T_sb[:], rhs=w2_sb[:], start=True,
                         stop=True)
        r2_sb = const.tile([2, C], F32)
        nc.scalar.copy(r2_sb[:], r2_ps[:])
        cal_ps = pmisc.tile([1, C], F32, space="PSUM", tag="mm")
        nc.tensor.matmul(cal_ps[:], lhsT=ones2[:], rhs=r2_sb[:], start=True,
                         stop=True)
        ca_sb = const.tile([1, C], F32)
        nc.scalar.activation(ca_sb[:], cal_ps[:], func=AF.Sigmoid)
        cab_ps = pmisc.tile([P, C], F32, space="PSUM", tag="mm")
        nc.tensor.matmul(cab_ps[:], lhsT=ones_row[:], rhs=ca_sb[:], start=True,
                         stop=True)
        cab = const.tile([P, C], F32)
        nc.scalar.copy(cab[:], cab_ps[:])

        # ---- phase 4: epilogue ----
        ph4 = ctx.enter_context(tc.tile_pool(name="ph4", bufs=1))
        hg_sb = ph4.tile([P, NB * C], F32)
        # bias already folded into agg via hslb; hg = ab * ca
        nc.vector.tensor_tensor(out=view3(hg_sb[:], NB, C),
                                in0=view3(agg_sb[:], NB, C),
                                in1=mid_bcast(cab[:], NB), op=ALU.mult)
        rsum = const.tile([P, NB], F32)
        nc.vector.reduce_sum(rsum[:], view3(hg_sb[:], NB, C),
                             axis=mybir.AxisListType.X)
        rmax = const.tile([P, NB], F32)
        nc.vector.reduce_max(rmax[:], view3(hg_sb[:], NB, C),
                             axis=mybir.AxisListType.X)
        t1 = const.tile([P, NB], F32)
        nc.vector.tensor_scalar(out=t1[:], in0=rsum[:], scalar1=sp_bc[:, 0:1],
                                scalar2=None, op0=ALU.mult)
        nc.vector.tensor_scalar(out=rmax[:], in0=rmax[:], scalar1=sp_bc[:, 1:2],
                                scalar2=None, op0=ALU.mult)
        nc.vector.tensor_tensor(out=t1[:], in0=t1[:], in1=rmax[:], op=ALU.add)
        sa = const.tile([P, NB], F32)
        nc.scalar.activation(sa[:], t1[:], func=AF.Sigmoid,
                             bias=sp_bc[:, 2:3], scale=1.0)
        nc.vector.tensor_tensor(out=view3(hg_sb[:], NB, C),
                                in0=view3(hg_sb[:], NB, C),
                                in1=sa[:].to_broadcast([P, NB, C]),
                                op=ALU.mult)
        nc.vector.tensor_tensor(out=view3(agg_sb[:], NB, C),
                                in0=view3(agg_sb[:], NB, C),
                                in1=view3(hg_sb[:], NB, C), op=ALU.add)
        nc.scalar.activation(agg_sb[:], agg_sb[:], func=AF.Relu)
        nc.sync.dma_start(out.ap().rearrange("(p b) c -> p b c", p=P),
                          view3(agg_sb[:], NB, C))

    nc.compile()
    return nc


def make_in_maps(inputs: dict, pp: dict, n_cores: int):
    NB, NSH, NPAD = pp["NB"], pp["NSH"], pp["NPAD"]
    x = np.asarray(inputs["x"], np.float32)
    N, C = x.shape
    x_pad = np.zeros((NPAD, C), np.float32)
    x_pad[:N] = x
    x_bf = x_pad.astype(ml_dtypes.bfloat16)
    sw = np.asarray(inputs["spatial_w"], np.float32)
    sb = np.asarray(inputs["spatial_b"], np.float32)
    sprow = np.array([[sw[0, 0] / C, sw[1, 0], sb[0]]], np.float32)
    W_bf = np.asarray(inputs["W"], np.float32).astype(ml_dtypes.bfloat16)

    in_maps = []
    for c in range(n_cores):
        in_maps.append({
            "xs": np.ascontiguousarray(x_bf[c * NSH:(c + 1) * NSH]),
            "idxw": np.ascontiguousarray(pp["idx_wraps"][c]),
            "idx32": np.ascontiguousarray(pp["idx32_arrs"][c]),
            "dstl": np.ascontiguousarray(
                pp["dstl_arrs"][c].astype(ml_dtypes.bfloat16)),
            "dinvd": np.ascontiguousarray(pp["dinv_dst"][c]),
            "W": W_bf,
            "brow": np.asarray(inputs["b"], np.float32).reshape(1, C),
            "w1": np.asarray(inputs["mlp_w1"], np.float32),
            "w2": np.asarray(inputs["mlp_w2"], np.float32),
            "sprow": sprow,
        })
    return in_maps


_CACHE = {}


def kernel(x, edge_index, W, b, mlp_w1, mlp_w2, spatial_w, spatial_b):
    inputs = {
        "x": np.asarray(x, np.float32),
        "edge_index": np.asarray(edge_index),
        "W": np.asarray(W, np.float32),
        "b": np.asarray(b, np.float32),
        "mlp_w1": np.asarray(mlp_w1, np.float32),
        "mlp_w2": np.asarray(mlp_w2, np.float32),
        "spatial_w": np.asarray(spatial_w, np.float32),
        "spatial_b": np.asarray(spatial_b, np.float32),
    }
    N, C = inputs["x"].shape
    H = inputs["mlp_w1"].shape[1]
    pp = preprocess(inputs["edge_index"], N, N_CORES)
    key = (N, C, H, pp["total_tiles"], pp["TTI"],
           tuple(tuple(sg[q][1:3] for q in range(4)) for sg in pp["sg_meta"]))
    if key not in _CACHE:
        _CACHE[key] = (build_nc(pp, N_CORES, N=N, C=C, H=H), pp["NSH"])
    nc, _ = _CACHE[key]
    in_maps = make_in_maps(inputs, pp, N_CORES)
    res = run_bass_kernel_spmd(nc, in_maps, list(range(N_CORES)))
    out = np.concatenate([res.results[c]["out"] for c in range(N_CORES)], 0)
    return np.ascontiguousarray(out[:N].astype(np.float32))

